# revision 1
# baseline (speedup 1.0000x reference)
"""GAT layer (nn_GATLayer) Trainium2 Bass kernel, 8-core SPMD row-sharded.

Per core (local rows m0..m0+1024 of the 8192-node graph):
  h  = X @ W, s1 = h @ a1, s2 = h @ a2
  e[m, j] = leaky_relu(s1[m] + s2[j], 0.2)
  att = softmax-masked(e) ; out = att @ h

Key algebraic restructure (removes the N x N exp/prelu passes):
  exp(lrelu(u)) = p * exp(u) + (1-p) * exp(a*u)        with p = [u >= 0]
               = p * E1_m * E2_j + (1-p) * F1_m * F2_j
  att_u = A . (p E1 E2 + (1-p) F1 F2)
  out_num = E1 . (M1 @ hE) + F1 . (M2 @ hF)
  M1 = A . p, M2 = A - M1, hE = E2.[h|1], hF = F2.[h|1]
  (at u==0 a blended p in (0,1) is still exact since E1E2 = F1F2 = 1)

p is built with a steep sigmoid on the scalar engine: sigmoid(k*u) with
k=256 rounds to exactly 0.0/1.0 in bf16 outside |u| < ~0.04, and inside
that band the blended value sits between the two branches which differ
by <2.5% there -- error well below the 2e-2 gate. GAT_PMODE=dve switches
to an exact fused add+is_ge on the vector engine instead.

All N x N traffic is bf16: A uploaded as bf16, masks bf16 (DVE 2x mode),
PE passes bf16 (1 cycle/row vs 4 for fp32). X^T and [W|w1|w2] are
precomputed host-side, so phase A has no transposes. DMA streams are
spread across queues: A-mask chunks on SP, X^T on gpsimd (engine queues
serialize a DMA's wire time with the issuing engine's compute).
"""

import os
import sys

sys.path.insert(0, "/opt/trn_rl_repo")

import numpy as np
import ml_dtypes

import concourse.bacc as bacc
import concourse.bass as bass
import concourse.tile as tile
from concourse import mybir
from concourse.alu_op_type import AluOpType
from concourse.bass_utils import run_bass_kernel_spmd

N, FIN, FOUT = 8192, 128, 64
NCORES = 8
MLOC = N // NCORES          # 1024 rows per core
NT = N // 128               # 64 j-tiles
MT = MLOC // 128            # 8 m-tiles
GRP = int(os.environ.get("GAT_GRP", "16"))  # j-tiles per phase-A h group
CHUNK = int(os.environ.get("GAT_CHUNK", "2"))   # j-tiles per A-mask chunk
ALPHA = 0.2
KSIG = float(os.environ.get("GAT_KSIG", "256.0"))

F32 = mybir.dt.float32
BF16 = mybir.dt.bfloat16
AF = mybir.ActivationFunctionType
BF = ml_dtypes.bfloat16


def build_kernel():
    nc = bacc.Bacc("TRN2", target_bir_lowering=False)

    xt_d = nc.dram_tensor("xt", (FIN, N), BF16, kind="ExternalInput")
    xtl_d = nc.dram_tensor("xt_loc", (FIN, MLOC), BF16, kind="ExternalInput")
    at_d = nc.dram_tensor("at_slab", (N, MLOC), BF16, kind="ExternalInput")
    wext_d = nc.dram_tensor("wext", (FIN, FOUT + 2), BF16, kind="ExternalInput")
    out_d = nc.dram_tensor("out", (MLOC, FOUT), F32, kind="ExternalOutput")
    eyef_d = nc.inline_tensor(np.eye(128, dtype=np.float32), "eyef")

    pmode = os.environ.get("GAT_PMODE", "sig")
    repeat = int(os.environ.get("GAT_REPEAT", "1"))
    nch = NT // CHUNK

    with tile.TileContext(nc) as tc:
        with (
            tc.tile_pool(name="const", bufs=1) as constp,
            tc.tile_pool(name="pers", bufs=1) as pers,
            tc.tile_pool(name="at", bufs=int(os.environ.get("GAT_ABUFS", "10"))) as atp,
            tc.tile_pool(name="pt", bufs=int(os.environ.get("GAT_PBUFS", "8"))) as ptp,
            tc.tile_pool(name="fin", bufs=3) as finp,
            tc.tile_pool(name="psEF", bufs=1, space="PSUM") as psEF,
            tc.tile_pool(name="psH", bufs=2 if GRP <= 8 else 1, space="PSUM") as psH,
            tc.tile_pool(name="psS", bufs=2, space="PSUM") as psS,
        ):
            # ---------------- constants / inputs ----------------
            wext = constp.tile([128, FOUT + 2], BF16)
            nc.sync.dma_start(out=wext, in_=wext_d.ap())
            xtl = constp.tile([128, MLOC], BF16)
            nc.sync.dma_start(out=xtl, in_=xtl_d.ap())
            ones1 = constp.tile([1, 128], F32)
            nc.vector.memset(ones1, 1.0)

            # A-mask chunk DMAs ride the otherwise-idle SP queue (the cost
            # model serializes a DMA's wire time on its issuing queue);
            # prefetch two chunks so the wire is busy from t=0.
            def issue_at_dma(eng=None):
                a8 = atp.tile([128, CHUNK, MLOC], BF16)
                g = issue_at_dma.next
                issue_at_dma.next = (g + 1) % nch
                src = bass.AP(
                    tensor=at_d, offset=g * CHUNK * 128 * MLOC,
                    ap=[[MLOC, 128], [128 * MLOC, CHUNK], [1, MLOC]],
                )
                (eng or nc.sync).dma_start(out=a8, in_=src)
                return a8

            issue_at_dma.next = 0
            # chunk 0 rides the act HWDGE queue (idle at t=0, ahead of the
            # consts serialized on SP) so the mask pipeline starts earliest
            at_q = [issue_at_dma(nc.scalar), issue_at_dma(), issue_at_dma()]
            # eyef is only needed by the finalize-side transposes, which are
            # issued after phase A -- load it behind the mask prefetch
            eyef = constp.tile([128, 128], F32)
            nc.sync.dma_start(out=eyef, in_=eyef_d.ap())

            # X^T in 4 slabs of 16 tiles on the gpsimd queue (idle early);
            # slab 0 first so phase A starts immediately, the rest after
            # the s1 section below.
            xt_sb = pers.tile([128, NT, 128], BF16)
            xt_view = xt_d.ap().rearrange("p (t i) -> p t i", i=128)

            def xt_slab(q):
                nc.gpsimd.dma_start(
                    out=xt_sb[:, q * 16:(q + 1) * 16, :],
                    in_=xt_view[:, q * 16:(q + 1) * 16, :],
                )

            xt_slab(0)

            # ---------------- s1 of local rows ----------------
            # s1locrow [1, 1024] = w1^T @ X_loc^T, two 512-halves (PSUM bank)
            s1locrow = pers.tile([1, MLOC], F32)
            for half in range(2):
                srp = psS.tile([1, 512], F32, tag="s")
                nc.tensor.matmul(
                    srp, lhsT=wext[:, FOUT:FOUT + 1],
                    rhs=xtl[:, half * 512:(half + 1) * 512],
                    start=True, stop=True,
                )
                nc.vector.tensor_copy(s1locrow[:, half * 512:(half + 1) * 512], srp)
            # s1repk [128, 1024] f32 = K*s1 replicated down partitions via
            # ones matmul; the K fold lets both p paths use K*s2 biases
            s1repk = pers.tile([128, MLOC], F32)
            for half in range(2):
                rp = psS.tile([128, 512], F32, tag="s")
                nc.tensor.matmul(
                    rp, lhsT=ones1,
                    rhs=s1locrow[:, half * 512:(half + 1) * 512],
                    start=True, stop=True,
                )
                nc.vector.tensor_scalar(
                    s1repk[:, half * 512:(half + 1) * 512], rp,
                    KSIG, None, AluOpType.mult)
            for q in range(1, 4):
                xt_slab(q)

            # ---------- phase A (h, E2/F2, hE/hF), then phase B ----------
            # p-mode: the first GAT_DVECH chunks use an exact fused
            # add+is_ge on DVE (no act-table dependency, instant start);
            # later chunks use the steep sigmoid on the act engine. All
            # phase-A exps are issued before any sigmoid, and engine queues
            # run in issue order, so the act table loads exactly twice.
            hE = pers.tile([128, NT, FOUT + 1], BF16)
            hF = pers.tile([128, NT, FOUT + 1], BF16)
            ks2all = pers.tile([128, NT], F32)
            ndve = int(os.environ.get("GAT_DVECH", "5"))
            if pmode != "sig":
                ndve = nch
            # chunks that keep A raw and stream M1 twice (PE 3-pass) to
            # offload the M2 = A - M1 subtraction from the DVE
            n3p = int(os.environ.get("GAT_N3P", "4"))
            pri = [c for c in range(3, nch - 1, 2)] + \
                  [c for c in range(2, nch - 1, 2)]
            chunks3p = set(pri[:n3p])
            # mid-stream chunks whose M2 = A - M1 runs on the idle gpsimd
            # engine (all-SBUF, so legal there); their psF matmuls are
            # deferred two chunk-slots so PE never waits on the slower
            # gpsimd op
            npool = int(os.environ.get("GAT_NPOOL", "12"))
            DEFER = int(os.environ.get("GAT_DEFER", "2"))
            pool_m2 = set(c for c in range(4, nch - 3, 2)
                          if c not in chunks3p)
            pool_m2 = set(sorted(pool_m2)[:npool])
            hFn = None
            if n3p:
                hFn = pers.tile([128, n3p * CHUNK, FOUT + 1], BF16, name="hFn")
            jt3p = {}
            for i, g in enumerate(sorted(chunks3p)):
                for k in range(CHUNK):
                    jt3p[g * CHUNK + k] = i * CHUNK + k

            psE = psEF.tile([FOUT + 1, MLOC], F32, tag="E")
            psF = psEF.tile([FOUT + 1, MLOC], F32, tag="F")
            total = repeat * nch
            totE = NT * 2
            totF = (NT + n3p * CHUNK) * 2
            for ga in range(NT // GRP):
                if True:
                    if True:  # phase A: build hE/hF for this group
                        g8 = ga * GRP
                        hb = psH.tile([128, GRP, FOUT], F32, tag="h")
                        s12 = psS.tile([128, GRP, 2], F32, tag="s")
                        for t in range(GRP):
                            jt = g8 + t
                            nc.tensor.matmul(
                                hb[:, t, :], lhsT=xt_sb[:, jt, :],
                                rhs=wext[:, 0:FOUT], start=True, stop=True)
                            nc.tensor.matmul(
                                s12[:, t, :], lhsT=xt_sb[:, jt, :],
                                rhs=wext[:, FOUT:FOUT + 2],
                                start=True, stop=True)
                        # E2 = exp(s2), F2 = exp(a*s2) (scalar engine,
                        # PSUM reads are legal on Act but not on gpsimd)
                        he2 = finp.tile([128, GRP, 1], F32)
                        hf2 = finp.tile([128, GRP, 1], F32)
                        nc.scalar.activation(he2, s12[:, :, 1:2], AF.Exp,
                                             bias=0.0, scale=1.0)
                        nc.scalar.activation(hf2, s12[:, :, 1:2], AF.Exp,
                                             bias=0.0, scale=ALPHA)
                        # stage h and s2 into SBUF so gpsimd can scale them
                        hbs = finp.tile([128, GRP, FOUT], BF16)
                        nc.scalar.activation(hbs, hb, AF.Copy,
                                             bias=0.0, scale=1.0)
                        s2s = finp.tile([128, GRP, 1], F32)
                        nc.vector.tensor_copy(s2s, s12[:, :, 1:2])
                        nc.gpsimd.tensor_scalar(
                            ks2all[:, g8:g8 + GRP].rearrange(
                                "p (g o) -> p g o", o=1),
                            s2s, KSIG, None, AluOpType.mult)
                        # hE/hF scale on gpsimd (otherwise idle)
                        for t in range(GRP):
                            jt = g8 + t
                            nc.gpsimd.tensor_scalar(
                                hE[:, jt, 0:FOUT], hbs[:, t, :], he2[:, t, :],
                                None, AluOpType.mult)
                            nc.gpsimd.tensor_scalar(
                                hF[:, jt, 0:FOUT], hbs[:, t, :], hf2[:, t, :],
                                None, AluOpType.mult)
                        nc.gpsimd.tensor_copy(
                            hE[:, g8:g8 + GRP, FOUT:FOUT + 1], he2)
                        nc.gpsimd.tensor_copy(
                            hF[:, g8:g8 + GRP, FOUT:FOUT + 1], hf2)
                        # negated hF for tiles in 3-pass chunks
                        n3_ts = [t for t in range(GRP) if g8 + t in jt3p]
                        if n3_ts:
                            hfn2 = finp.tile([128, GRP, 1], F32)
                            nc.gpsimd.tensor_scalar(
                                hfn2, hf2, -1.0, None, AluOpType.mult)
                            for t in n3_ts:
                                jn = jt3p[g8 + t]
                                nc.gpsimd.tensor_scalar(
                                    hFn[:, jn, 0:FOUT], hbs[:, t, :],
                                    hfn2[:, t, :], None, AluOpType.mult)
                                nc.gpsimd.tensor_copy(
                                    hFn[:, jn, FOUT:FOUT + 1],
                                    hfn2[:, t, :])

            # E1/F1 columns for the finalize (issued after phase A so the
            # act/PE queue heads are not blocked on eyef early on)
            psT = psS.tile([128, MT], F32, tag="s")
            for t in range(MT):
                nc.tensor.transpose(
                    psT[:, t:t + 1], s1locrow[:, t * 128:(t + 1) * 128],
                    eyef[0:1, 0:1],
                )
            e1all = pers.tile([128, MT], F32)
            f1all = pers.tile([128, MT], F32)
            nc.scalar.activation(e1all, psT, AF.Exp, bias=0.0, scale=1.0)
            nc.scalar.activation(f1all, psT, AF.Exp, bias=0.0, scale=ALPHA)

            for step in range(total):
                g = step % nch
                if g == 0:
                    cntE = [0, 0]
                    cntF = [0, 0]
                a8 = at_q[step]
                if step + 3 < total:
                    at_q.append(issue_at_dma())
                p8 = ptp.tile([128, CHUNK * MLOC], BF16)
                for k in range(CHUNK):
                    jt = g * CHUNK + k
                    if g < ndve:
                        nc.vector.tensor_scalar(
                            p8[:, k * MLOC:(k + 1) * MLOC], s1repk,
                            ks2all[:, jt:jt + 1], 0.0,
                            AluOpType.add, AluOpType.is_ge)
                    else:
                        nc.scalar.activation(
                            p8[:, k * MLOC:(k + 1) * MLOC], s1repk,
                            AF.Sigmoid, bias=ks2all[:, jt:jt + 1],
                            scale=1.0)
                af = a8.rearrange("p c m -> p (c m)")

                def mm(ps, cnt, tot, half, lhsT, rhs):
                    nc.tensor.matmul(
                        ps[:, half * 512:(half + 1) * 512], lhsT=lhsT,
                        rhs=rhs, start=cnt[half] == 0,
                        stop=cnt[half] == tot - 2,
                    )
                    cnt[half] += 2

                # M1 = A . p (in place on p8); M2 = A - M1 (in place on a8)
                # except on 3-pass chunks where A stays raw. The last chunk
                # runs per-tile so the finalize can start sooner.
                if g == 0:
                    deferred = {}
                per_tile = step == total - 1
                for k0 in (range(CHUNK) if per_tile else [0]):
                    kw = 1 if per_tile else CHUNK
                    slc = slice(k0 * MLOC, (k0 + kw) * MLOC)
                    nc.vector.tensor_tensor(
                        p8[:, slc], p8[:, slc], af[:, slc], AluOpType.mult)
                    if g not in chunks3p:
                        eng = nc.gpsimd if g in pool_m2 else nc.vector
                        eng.tensor_tensor(
                            af[:, slc], af[:, slc], p8[:, slc],
                            AluOpType.subtract)
                    for k in range(k0, k0 + kw):
                        jt = g * CHUNK + k
                        for half in range(2):
                            sl = slice(k * MLOC + half * 512,
                                       k * MLOC + half * 512 + 512)
                            mm(psE, cntE, totE, half, hE[:, jt, :], p8[:, sl])
                            if g in pool_m2:
                                deferred.setdefault(g + DEFER, []).append(
                                    (half, hF[:, jt, :], af[:, sl]))
                            else:
                                mm(psF, cntF, totF, half,
                                   hF[:, jt, :], af[:, sl])
                            if g in chunks3p:
                                mm(psF, cntF, totF, half,
                                   hFn[:, jt3p[jt], :], p8[:, sl])
                for half, lh, rh in deferred.pop(g, []):
                    mm(psF, cntF, totF, half, lh, rh)

            # ---------------- finalize ----------------
            # GAT_FIN=dma: accumulators go PSUM -> bf16 SBUF (padded to
            # 80 rows for the XBAR) and DMA-engine transposes replace the
            # PE transposes. Measured no better than the PE path (each
            # dma_start_transpose pays ~630 ns HWDGE issue overhead), so
            # the PE path stays the default.
            fmode = os.environ.get("GAT_FIN", "ps")
            if fmode == "dma":
                oEb = pers.tile([80, MLOC], BF16)
                oFb = pers.tile([80, MLOC], BF16)
                nc.vector.memset(oEb, 0.0)
                nc.vector.memset(oFb, 0.0)
                nc.vector.tensor_copy(oEb[0:FOUT + 1, :], psE)
                nc.scalar.activation(oFb[0:FOUT + 1, :], psF, AF.Copy,
                                     bias=0.0, scale=1.0)
                oEt = pers.tile([128, MT, 80], BF16)
                oFt = pers.tile([128, MT, 80], BF16)
                for t in range(MT):
                    nc.sync.dma_start_transpose(
                        oEt[:, t, :], oEb[:, t * 128:(t + 1) * 128])
                    nc.scalar.dma_start_transpose(
                        oFt[:, t, :], oFb[:, t * 128:(t + 1) * 128])
                for t in range(MT):
                    na = finp.tile([128, FOUT + 1], F32)
                    nb = finp.tile([128, FOUT + 1], F32)
                    nc.vector.tensor_scalar(
                        na, oEt[:, t, 0:FOUT + 1], e1all[:, t:t + 1], None,
                        AluOpType.mult)
                    nc.scalar.activation(nb, oFt[:, t, 0:FOUT + 1], AF.Copy,
                                         bias=0.0, scale=f1all[:, t:t + 1])
                    nc.vector.tensor_tensor(na, na, nb, AluOpType.add)
                    rec = finp.tile([128, 1], F32)
                    nc.vector.reciprocal(rec, na[:, FOUT:FOUT + 1])
                    fin = finp.tile([128, FOUT], F32)
                    nc.vector.tensor_scalar(fin, na[:, 0:FOUT], rec, None,
                                            AluOpType.mult)
                    nc.sync.dma_start(
                        out=out_d.ap()[t * 128:(t + 1) * 128, :], in_=fin)
            else:
                oE = pers.tile([FOUT + 1, MLOC], F32)
                oF = pers.tile([FOUT + 1, MLOC], F32)
                nc.vector.tensor_copy(oE, psE)
                nc.scalar.activation(oF, psF, AF.Copy, bias=0.0, scale=1.0)
                for t in range(MT):
                    trE = psS.tile([128, FOUT + 1], F32, tag="s")
                    trF = psH.tile([128, FOUT + 1], F32, tag="h")
                    nc.tensor.transpose(
                        trE, oE[:, t * 128:(t + 1) * 128],
                        eyef[0:FOUT + 1, 0:FOUT + 1])
                    nc.tensor.transpose(
                        trF, oF[:, t * 128:(t + 1) * 128],
                        eyef[0:FOUT + 1, 0:FOUT + 1])
                    na = finp.tile([128, FOUT + 1], F32)
                    nb = finp.tile([128, FOUT + 1], F32)
                    nc.vector.tensor_scalar(na, trE, e1all[:, t:t + 1], None,
                                            AluOpType.mult)
                    nc.scalar.activation(nb, trF, AF.Copy, bias=0.0,
                                         scale=f1all[:, t:t + 1])
                    nc.vector.tensor_tensor(na, na, nb, AluOpType.add)
                    rec = finp.tile([128, 1], F32)
                    nc.vector.reciprocal(rec, na[:, FOUT:FOUT + 1])
                    fin = finp.tile([128, FOUT], F32)
                    nc.vector.tensor_scalar(fin, na[:, 0:FOUT], rec, None,
                                            AluOpType.mult)
                    nc.sync.dma_start(
                        out=out_d.ap()[t * 128:(t + 1) * 128, :], in_=fin)

    nc.compile()
    return nc


_NC = None


def kernel(X, A, W, a1, a2):
    global _NC
    X = np.asarray(X, dtype=np.float32)
    A = np.asarray(A)
    W = np.asarray(W, dtype=np.float32)
    a1 = np.asarray(a1, dtype=np.float32)
    a2 = np.asarray(a2, dtype=np.float32)

    xt = np.ascontiguousarray(X.T).astype(BF)
    wext = np.ascontiguousarray(
        np.concatenate([W, (W @ a1)[:, None], (W @ a2)[:, None]], axis=1)
    ).astype(BF)
    A_bf = A.astype(BF)

    if _NC is None:
        _NC = build_kernel()
    nc = _NC
    in_maps = []
    for c in range(NCORES):
        rows = slice(c * MLOC, (c + 1) * MLOC)
        in_maps.append({
            "xt": xt,
            "xt_loc": np.ascontiguousarray(xt[:, rows]),
            "at_slab": np.ascontiguousarray(A_bf[rows].T),
            "wext": wext,
        })
    res = run_bass_kernel_spmd(nc, in_maps, core_ids=list(range(NCORES)))
    return np.concatenate([r["out"] for r in res.results], axis=0)


if __name__ == "__main__":
    rng = np.random.default_rng(0)
    X = rng.standard_normal((N, FIN), dtype=np.float32)
    A = rng.integers(0, 2, (N, N), dtype=np.int32)
    W = (rng.standard_normal((FIN, FOUT), dtype=np.float32) * 0.05)
    a1 = (rng.standard_normal((FOUT,), dtype=np.float32) * 0.05)
    a2 = (rng.standard_normal((FOUT,), dtype=np.float32) * 0.05)
    out = kernel(X=X, A=A, W=W, a1=a1, a2=a2)
    h = X @ W
    s1 = h @ a1
    s2 = h @ a2
    e = s1[:, None] + s2[None, :]
    e = np.where(e > 0, e, ALPHA * e)
    att = np.where(A > 0, np.exp(e - e.max(1, keepdims=True)), 0.0)
    att = att / att.sum(1, keepdims=True)
    ref = att @ h
    err = np.abs(out - ref).max() / np.abs(ref).max()
    print("rel err (max-abs scaled):", err)



# revision 2
# speedup vs baseline: 1.2590x; 1.2590x over previous
"""GAT layer (nn_GATLayer) Trainium2 Bass kernel, 8-core SPMD row-sharded.

Per core (local rows m0..m0+1024 of the 8192-node graph):
  h  = X @ W, s1 = h @ a1, s2 = h @ a2
  e[m, j] = leaky_relu(s1[m] + s2[j], 0.2)
  att = softmax-masked(e) ; out = att @ h

Key algebraic restructure (removes the N x N exp/prelu passes):
  exp(lrelu(u)) = p * exp(u) + (1-p) * exp(a*u)        with p = [u >= 0]
               = p * E1_m * E2_j + (1-p) * F1_m * F2_j
  att_u = A . (p E1 E2 + (1-p) F1 F2)
  out_num = E1 . (M1 @ hE) + F1 . (M2 @ hF)
  M1 = A . p, M2 = A - M1, hE = E2.[h|1], hF = F2.[h|1]
  (at u==0 a blended p in (0,1) is still exact since E1E2 = F1F2 = 1)

p is built with a steep sigmoid on the scalar engine: sigmoid(k*u) with
k=256 rounds to exactly 0.0/1.0 in bf16 outside |u| < ~0.04, and inside
that band the blended value sits between the two branches which differ
by <2.5% there -- error well below the 2e-2 gate. GAT_PMODE=dve switches
to an exact fused add+is_ge on the vector engine instead.

All N x N traffic is bf16: A uploaded as bf16, masks bf16 (DVE 2x mode),
PE passes bf16 (1 cycle/row vs 4 for fp32). X^T and [W|w1|w2] are
precomputed host-side, so phase A has no transposes. DMA streams are
spread across queues: A-mask chunks on SP, X^T on gpsimd (engine queues
serialize a DMA's wire time with the issuing engine's compute).
"""

import os
import sys

sys.path.insert(0, "/opt/trn_rl_repo")

import numpy as np
import ml_dtypes

import concourse.bacc as bacc
import concourse.bass as bass
import concourse.tile as tile
from concourse import mybir
from concourse.alu_op_type import AluOpType
from concourse.bass_utils import run_bass_kernel_spmd

N, FIN, FOUT = 8192, 128, 64
NCORES = 8
MLOC = N // NCORES          # 1024 rows per core
NT = N // 128               # 64 j-tiles
MT = MLOC // 128            # 8 m-tiles
GRP = int(os.environ.get("GAT_GRP", "16"))  # j-tiles per phase-A h group
CHUNK = int(os.environ.get("GAT_CHUNK", "2"))   # j-tiles per A-mask chunk
ALPHA = 0.2
KSIG = float(os.environ.get("GAT_KSIG", "256.0"))

F32 = mybir.dt.float32
BF16 = mybir.dt.bfloat16
AF = mybir.ActivationFunctionType
BF = ml_dtypes.bfloat16


def build_kernel():
    nc = bacc.Bacc("TRN2", target_bir_lowering=False)

    xt_d = nc.dram_tensor("xt", (FIN, N), BF16, kind="ExternalInput")
    xtl_d = nc.dram_tensor("xt_loc", (FIN, MLOC), BF16, kind="ExternalInput")
    at_d = nc.dram_tensor("at_slab", (N, MLOC), BF16, kind="ExternalInput")
    wext_d = nc.dram_tensor("wext", (FIN, FOUT + 2), BF16, kind="ExternalInput")
    out_d = nc.dram_tensor("out", (MLOC, FOUT), F32, kind="ExternalOutput")
    eyef_d = nc.inline_tensor(np.eye(128, dtype=np.float32), "eyef")

    pmode = os.environ.get("GAT_PMODE", "sig")
    repeat = int(os.environ.get("GAT_REPEAT", "1"))
    nch = NT // CHUNK

    with tile.TileContext(nc) as tc:
        with (
            tc.tile_pool(name="const", bufs=1) as constp,
            tc.tile_pool(name="pers", bufs=1) as pers,
            tc.tile_pool(name="at", bufs=int(os.environ.get("GAT_ABUFS", "10"))) as atp,
            tc.tile_pool(name="pt", bufs=int(os.environ.get("GAT_PBUFS", "8"))) as ptp,
            tc.tile_pool(name="fin", bufs=3) as finp,
            tc.tile_pool(name="psEF", bufs=1, space="PSUM") as psEF,
            tc.tile_pool(name="psH", bufs=2 if GRP <= 8 else 1, space="PSUM") as psH,
            tc.tile_pool(name="psS", bufs=2, space="PSUM") as psS,
        ):
            # ---------------- constants / inputs ----------------
            wext = constp.tile([128, FOUT + 2], BF16)
            nc.sync.dma_start(out=wext, in_=wext_d.ap())
            xtl = constp.tile([128, MLOC], BF16)
            nc.sync.dma_start(out=xtl, in_=xtl_d.ap())
            ones1 = constp.tile([1, 128], F32)
            nc.vector.memset(ones1, 1.0)

            # A-mask chunk DMAs ride the otherwise-idle SP queue (the cost
            # model serializes a DMA's wire time on its issuing queue);
            # prefetch two chunks so the wire is busy from t=0.
            def issue_at_dma(eng=None):
                a8 = atp.tile([128, CHUNK, MLOC], BF16)
                g = issue_at_dma.next
                issue_at_dma.next = (g + 1) % nch
                src = bass.AP(
                    tensor=at_d, offset=g * CHUNK * 128 * MLOC,
                    ap=[[MLOC, 128], [128 * MLOC, CHUNK], [1, MLOC]],
                )
                (eng or nc.sync).dma_start(out=a8, in_=src)
                return a8

            issue_at_dma.next = 0
            # chunk 0 rides the act HWDGE queue (idle at t=0, ahead of the
            # consts serialized on SP) so the mask pipeline starts earliest
            at_q = [issue_at_dma(nc.scalar), issue_at_dma(), issue_at_dma()]
            # eyef is only needed by the finalize-side transposes, which are
            # issued after phase A -- load it behind the mask prefetch
            eyef = constp.tile([128, 128], F32)
            nc.sync.dma_start(out=eyef, in_=eyef_d.ap())

            # X^T in 4 slabs of 16 tiles on the gpsimd queue (idle early);
            # slab 0 first so phase A starts immediately, the rest after
            # the s1 section below.
            xt_sb = pers.tile([128, NT, 128], BF16)
            xt_view = xt_d.ap().rearrange("p (t i) -> p t i", i=128)

            def xt_slab(q):
                nc.gpsimd.dma_start(
                    out=xt_sb[:, q * 16:(q + 1) * 16, :],
                    in_=xt_view[:, q * 16:(q + 1) * 16, :],
                )

            xt_slab(0)

            # ---------------- s1 of local rows ----------------
            # s1locrow [1, 1024] = w1^T @ X_loc^T, two 512-halves (PSUM bank)
            s1locrow = pers.tile([1, MLOC], F32)
            for half in range(2):
                srp = psS.tile([1, 512], F32, tag="s")
                nc.tensor.matmul(
                    srp, lhsT=wext[:, FOUT:FOUT + 1],
                    rhs=xtl[:, half * 512:(half + 1) * 512],
                    start=True, stop=True,
                )
                nc.vector.tensor_copy(s1locrow[:, half * 512:(half + 1) * 512], srp)
            # s1repk [128, 1024] f32 = K*s1 replicated down partitions via
            # ones matmul; the K fold lets both p paths use K*s2 biases
            s1repk = pers.tile([128, MLOC], F32)
            for half in range(2):
                rp = psS.tile([128, 512], F32, tag="s")
                nc.tensor.matmul(
                    rp, lhsT=ones1,
                    rhs=s1locrow[:, half * 512:(half + 1) * 512],
                    start=True, stop=True,
                )
                nc.vector.tensor_scalar(
                    s1repk[:, half * 512:(half + 1) * 512], rp,
                    KSIG, None, AluOpType.mult)
            for q in range(1, 4):
                xt_slab(q)

            # ---------- phase A (h, E2/F2, hE/hF), then phase B ----------
            # p-mode: the first GAT_DVECH chunks use an exact fused
            # add+is_ge on DVE (no act-table dependency, instant start);
            # later chunks use the steep sigmoid on the act engine. All
            # phase-A exps are issued before any sigmoid, and engine queues
            # run in issue order, so the act table loads exactly twice.
            hE = pers.tile([128, NT, FOUT + 1], BF16)
            hF = pers.tile([128, NT, FOUT + 1], BF16)
            ks2all = pers.tile([128, NT], F32)
            ndve = int(os.environ.get("GAT_DVECH", "5"))
            if pmode != "sig":
                ndve = nch
            # chunks that keep A raw and stream M1 twice (PE 3-pass) to
            # offload the M2 = A - M1 subtraction from the DVE
            n3p = int(os.environ.get("GAT_N3P", "4"))
            pri = [c for c in range(3, nch - 1, 2)] + \
                  [c for c in range(2, nch - 1, 2)]
            chunks3p = set(pri[:n3p])
            # mid-stream chunks whose M2 = A - M1 runs on the idle gpsimd
            # engine (all-SBUF, so legal there); their psF matmuls are
            # deferred two chunk-slots so PE never waits on the slower
            # gpsimd op
            npool = int(os.environ.get("GAT_NPOOL", "12"))
            DEFER = int(os.environ.get("GAT_DEFER", "2"))
            pool_m2 = set(c for c in range(4, nch - 3, 2)
                          if c not in chunks3p)
            pool_m2 = set(sorted(pool_m2)[:npool])
            hFn = None
            if n3p:
                hFn = pers.tile([128, n3p * CHUNK, FOUT + 1], BF16, name="hFn")
            jt3p = {}
            for i, g in enumerate(sorted(chunks3p)):
                for k in range(CHUNK):
                    jt3p[g * CHUNK + k] = i * CHUNK + k

            psE = psEF.tile([FOUT + 1, MLOC], F32, tag="E")
            psF = psEF.tile([FOUT + 1, MLOC], F32, tag="F")
            total = repeat * nch
            totE = NT * 2
            totF = (NT + n3p * CHUNK) * 2
            for ga in range(NT // GRP):
                if True:
                    if True:  # phase A: build hE/hF for this group
                        g8 = ga * GRP
                        hb = psH.tile([128, GRP, FOUT], F32, tag="h")
                        s12 = psS.tile([128, GRP, 2], F32, tag="s")
                        for t in range(GRP):
                            jt = g8 + t
                            nc.tensor.matmul(
                                hb[:, t, :], lhsT=xt_sb[:, jt, :],
                                rhs=wext[:, 0:FOUT], start=True, stop=True)
                            nc.tensor.matmul(
                                s12[:, t, :], lhsT=xt_sb[:, jt, :],
                                rhs=wext[:, FOUT:FOUT + 2],
                                start=True, stop=True)
                        # E2 = exp(s2), F2 = exp(a*s2) (scalar engine,
                        # PSUM reads are legal on Act but not on gpsimd)
                        he2 = finp.tile([128, GRP, 1], F32)
                        hf2 = finp.tile([128, GRP, 1], F32)
                        nc.scalar.activation(he2, s12[:, :, 1:2], AF.Exp,
                                             bias=0.0, scale=1.0)
                        nc.scalar.activation(hf2, s12[:, :, 1:2], AF.Exp,
                                             bias=0.0, scale=ALPHA)
                        # stage h and s2 into SBUF so gpsimd can scale them
                        hbs = finp.tile([128, GRP, FOUT], BF16)
                        nc.scalar.activation(hbs, hb, AF.Copy,
                                             bias=0.0, scale=1.0)
                        s2s = finp.tile([128, GRP, 1], F32)
                        nc.vector.tensor_copy(s2s, s12[:, :, 1:2])
                        nc.gpsimd.tensor_scalar(
                            ks2all[:, g8:g8 + GRP].rearrange(
                                "p (g o) -> p g o", o=1),
                            s2s, KSIG, None, AluOpType.mult)
                        # hE/hF scale on gpsimd (otherwise idle)
                        for t in range(GRP):
                            jt = g8 + t
                            nc.gpsimd.tensor_scalar(
                                hE[:, jt, 0:FOUT], hbs[:, t, :], he2[:, t, :],
                                None, AluOpType.mult)
                            nc.gpsimd.tensor_scalar(
                                hF[:, jt, 0:FOUT], hbs[:, t, :], hf2[:, t, :],
                                None, AluOpType.mult)
                        nc.gpsimd.tensor_copy(
                            hE[:, g8:g8 + GRP, FOUT:FOUT + 1], he2)
                        nc.gpsimd.tensor_copy(
                            hF[:, g8:g8 + GRP, FOUT:FOUT + 1], hf2)
                        # negated hF for tiles in 3-pass chunks
                        n3_ts = [t for t in range(GRP) if g8 + t in jt3p]
                        if n3_ts:
                            hfn2 = finp.tile([128, GRP, 1], F32)
                            nc.gpsimd.tensor_scalar(
                                hfn2, hf2, -1.0, None, AluOpType.mult)
                            for t in n3_ts:
                                jn = jt3p[g8 + t]
                                nc.gpsimd.tensor_scalar(
                                    hFn[:, jn, 0:FOUT], hbs[:, t, :],
                                    hfn2[:, t, :], None, AluOpType.mult)
                                nc.gpsimd.tensor_copy(
                                    hFn[:, jn, FOUT:FOUT + 1],
                                    hfn2[:, t, :])

            # E1/F1 columns for the finalize (issued after phase A so the
            # act/PE queue heads are not blocked on eyef early on)
            psT = psS.tile([128, MT], F32, tag="s")
            for t in range(MT):
                nc.tensor.transpose(
                    psT[:, t:t + 1], s1locrow[:, t * 128:(t + 1) * 128],
                    eyef[0:1, 0:1],
                )
            e1all = pers.tile([128, MT], F32)
            f1all = pers.tile([128, MT], F32)
            nc.scalar.activation(e1all, psT, AF.Exp, bias=0.0, scale=1.0)
            nc.scalar.activation(f1all, psT, AF.Exp, bias=0.0, scale=ALPHA)

            for step in range(total):
                g = step % nch
                if g == 0:
                    cntE = [0, 0]
                    cntF = [0, 0]
                a8 = at_q[step]
                if step + 3 < total:
                    at_q.append(issue_at_dma())
                p8 = ptp.tile([128, CHUNK * MLOC], BF16)
                for k in range(CHUNK):
                    jt = g * CHUNK + k
                    if g < ndve:
                        nc.vector.tensor_scalar(
                            p8[:, k * MLOC:(k + 1) * MLOC], s1repk,
                            ks2all[:, jt:jt + 1], 0.0,
                            AluOpType.add, AluOpType.is_ge)
                    else:
                        nc.scalar.activation(
                            p8[:, k * MLOC:(k + 1) * MLOC], s1repk,
                            AF.Sigmoid, bias=ks2all[:, jt:jt + 1],
                            scale=1.0)
                af = a8.rearrange("p c m -> p (c m)")

                NOF = os.environ.get("GAT_NOF") == "1"

                def mm(ps, cnt, tot, half, lhsT, rhs):
                    if NOF and ps is psF:
                        return
                    nc.tensor.matmul(
                        ps[:, half * 512:(half + 1) * 512], lhsT=lhsT,
                        rhs=rhs, start=cnt[half] == 0,
                        stop=cnt[half] == tot - 2,
                    )
                    cnt[half] += 2

                # M1 = A . p (in place on p8); M2 = A - M1 (in place on a8)
                # except on 3-pass chunks where A stays raw. The last chunk
                # runs per-tile so the finalize can start sooner.
                if g == 0:
                    deferred = {}
                per_tile = step == total - 1
                for k0 in (range(CHUNK) if per_tile else [0]):
                    kw = 1 if per_tile else CHUNK
                    slc = slice(k0 * MLOC, (k0 + kw) * MLOC)
                    nc.vector.tensor_tensor(
                        p8[:, slc], p8[:, slc], af[:, slc], AluOpType.mult)
                    if g not in chunks3p and os.environ.get("GAT_NOF") != "1":
                        eng = nc.gpsimd if g in pool_m2 else nc.vector
                        eng.tensor_tensor(
                            af[:, slc], af[:, slc], p8[:, slc],
                            AluOpType.subtract)
                    for k in range(k0, k0 + kw):
                        jt = g * CHUNK + k
                        for half in range(2):
                            sl = slice(k * MLOC + half * 512,
                                       k * MLOC + half * 512 + 512)
                            mm(psE, cntE, totE, half, hE[:, jt, :], p8[:, sl])
                            if g in pool_m2:
                                deferred.setdefault(g + DEFER, []).append(
                                    (half, hF[:, jt, :], af[:, sl]))
                            else:
                                mm(psF, cntF, totF, half,
                                   hF[:, jt, :], af[:, sl])
                            if g in chunks3p:
                                mm(psF, cntF, totF, half,
                                   hFn[:, jt3p[jt], :], p8[:, sl])
                for half, lh, rh in deferred.pop(g, []):
                    mm(psF, cntF, totF, half, lh, rh)

            # ---------------- finalize ----------------
            # GAT_FIN=dma: accumulators go PSUM -> bf16 SBUF (padded to
            # 80 rows for the XBAR) and DMA-engine transposes replace the
            # PE transposes. Measured no better than the PE path (each
            # dma_start_transpose pays ~630 ns HWDGE issue overhead), so
            # the PE path stays the default.
            fmode = os.environ.get("GAT_FIN", "ps")
            if fmode == "dma":
                oEb = pers.tile([80, MLOC], BF16)
                oFb = pers.tile([80, MLOC], BF16)
                nc.vector.memset(oEb, 0.0)
                nc.vector.memset(oFb, 0.0)
                nc.vector.tensor_copy(oEb[0:FOUT + 1, :], psE)
                nc.scalar.activation(oFb[0:FOUT + 1, :], psF, AF.Copy,
                                     bias=0.0, scale=1.0)
                oEt = pers.tile([128, MT, 80], BF16)
                oFt = pers.tile([128, MT, 80], BF16)
                for t in range(MT):
                    nc.sync.dma_start_transpose(
                        oEt[:, t, :], oEb[:, t * 128:(t + 1) * 128])
                    nc.scalar.dma_start_transpose(
                        oFt[:, t, :], oFb[:, t * 128:(t + 1) * 128])
                for t in range(MT):
                    na = finp.tile([128, FOUT + 1], F32)
                    nb = finp.tile([128, FOUT + 1], F32)
                    nc.vector.tensor_scalar(
                        na, oEt[:, t, 0:FOUT + 1], e1all[:, t:t + 1], None,
                        AluOpType.mult)
                    nc.scalar.activation(nb, oFt[:, t, 0:FOUT + 1], AF.Copy,
                                         bias=0.0, scale=f1all[:, t:t + 1])
                    nc.vector.tensor_tensor(na, na, nb, AluOpType.add)
                    rec = finp.tile([128, 1], F32)
                    nc.vector.reciprocal(rec, na[:, FOUT:FOUT + 1])
                    fin = finp.tile([128, FOUT], F32)
                    nc.vector.tensor_scalar(fin, na[:, 0:FOUT], rec, None,
                                            AluOpType.mult)
                    nc.sync.dma_start(
                        out=out_d.ap()[t * 128:(t + 1) * 128, :], in_=fin)
            else:
                oE = pers.tile([FOUT + 1, MLOC], F32)
                oF = pers.tile([FOUT + 1, MLOC], F32)
                nc.vector.tensor_copy(oE, psE)
                nc.scalar.activation(oF, psF, AF.Copy, bias=0.0, scale=1.0)
                for t in range(MT):
                    trE = psS.tile([128, FOUT + 1], F32, tag="s")
                    trF = psH.tile([128, FOUT + 1], F32, tag="h")
                    nc.tensor.transpose(
                        trE, oE[:, t * 128:(t + 1) * 128],
                        eyef[0:FOUT + 1, 0:FOUT + 1])
                    nc.tensor.transpose(
                        trF, oF[:, t * 128:(t + 1) * 128],
                        eyef[0:FOUT + 1, 0:FOUT + 1])
                    na = finp.tile([128, FOUT + 1], F32)
                    nb = finp.tile([128, FOUT + 1], F32)
                    nc.vector.tensor_scalar(na, trE, e1all[:, t:t + 1], None,
                                            AluOpType.mult)
                    nc.scalar.activation(nb, trF, AF.Copy, bias=0.0,
                                         scale=f1all[:, t:t + 1])
                    nc.vector.tensor_tensor(na, na, nb, AluOpType.add)
                    rec = finp.tile([128, 1], F32)
                    nc.vector.reciprocal(rec, na[:, FOUT:FOUT + 1])
                    fin = finp.tile([128, FOUT], F32)
                    nc.vector.tensor_scalar(fin, na[:, 0:FOUT], rec, None,
                                            AluOpType.mult)
                    nc.sync.dma_start(
                        out=out_d.ap()[t * 128:(t + 1) * 128, :], in_=fin)

    nc.compile()
    return nc


_NC = None


def kernel(X, A, W, a1, a2):
    global _NC
    X = np.asarray(X, dtype=np.float32)
    A = np.asarray(A)
    W = np.asarray(W, dtype=np.float32)
    a1 = np.asarray(a1, dtype=np.float32)
    a2 = np.asarray(a2, dtype=np.float32)

    xt = np.ascontiguousarray(X.T).astype(BF)
    wext = np.ascontiguousarray(
        np.concatenate([W, (W @ a1)[:, None], (W @ a2)[:, None]], axis=1)
    ).astype(BF)
    A_bf = A.astype(BF)

    if _NC is None:
        _NC = build_kernel()
    nc = _NC
    in_maps = []
    for c in range(NCORES):
        rows = slice(c * MLOC, (c + 1) * MLOC)
        in_maps.append({
            "xt": xt,
            "xt_loc": np.ascontiguousarray(xt[:, rows]),
            "at_slab": np.ascontiguousarray(A_bf[rows].T),
            "wext": wext,
        })
    res = run_bass_kernel_spmd(nc, in_maps, core_ids=list(range(NCORES)))
    return np.concatenate([r["out"] for r in res.results], axis=0)


if __name__ == "__main__":
    rng = np.random.default_rng(0)
    X = rng.standard_normal((N, FIN), dtype=np.float32)
    A = rng.integers(0, 2, (N, N), dtype=np.int32)
    W = (rng.standard_normal((FIN, FOUT), dtype=np.float32) * 0.05)
    a1 = (rng.standard_normal((FOUT,), dtype=np.float32) * 0.05)
    a2 = (rng.standard_normal((FOUT,), dtype=np.float32) * 0.05)
    out = kernel(X=X, A=A, W=W, a1=a1, a2=a2)
    h = X @ W
    s1 = h @ a1
    s2 = h @ a2
    e = s1[:, None] + s2[None, :]
    e = np.where(e > 0, e, ALPHA * e)
    att = np.where(A > 0, np.exp(e - e.max(1, keepdims=True)), 0.0)
    att = att / att.sum(1, keepdims=True)
    ref = att @ h
    err = np.abs(out - ref).max() / np.abs(ref).max()
    print("rel err (max-abs scaled):", err)



# revision 3
# speedup vs baseline: 1.4154x; 1.1243x over previous
"""GAT layer (nn_GATLayer) Trainium2 Bass kernel, 8-core SPMD row-sharded.

Direct-U formulation. Per core (local rows m0..m0+1024 of the 8192 graph):
  att_u[m, j] = A[m,j] * exp(leaky_relu(s1[m] + s2[j], 0.2))
  out = softmax-normalized att_u @ h

Key identities (all exact):
  exp(leaky_relu(u)) = max(e^u, e^{a u})            for a in [0, 1]
  att_u = E1(m) * [ A . E2(j) . max(G1(m) G2(j), 1) ]
      with E = e^{s}, G = e^{(a-1)s};  E1(m) cancels in the softmax ratio.

So per j-tile the device only computes
  U = (G1rep * g2[j]) max 1        -- tensor_scalar, 4x DVE mode (bf16 SBUF)
  R = A . U                        -- tensor_tensor, 2x DVE / gpsimd split
  psEF += (E2.[h|1])^T R           -- ONE matmul accumulation pass
and the finalize divides numerator rows by the denominator row. This is one
PE pass + two elementwise passes vs the E/F-split baseline's two PE passes +
three elementwise passes, with no sigmoid/blend approximation at all.

All N x N traffic is bf16. X^T and [W|w1|w2] are precomputed host-side.
A-chunk DMAs alternate between the SP and Act hardware queues (the act
engine only has small phase-A work here); X^T rides the gpsimd queue.
"""

import os
import sys

sys.path.insert(0, "/opt/trn_rl_repo")

import numpy as np
import ml_dtypes

import concourse.bacc as bacc
import concourse.bass as bass
import concourse.tile as tile
from concourse import mybir
from concourse.alu_op_type import AluOpType
from concourse.bass_utils import run_bass_kernel_spmd

N, FIN, FOUT = 8192, 128, 64
NCORES = 8
MLOC = N // NCORES          # 1024 rows per core
NT = N // 128               # 64 j-tiles
MT = MLOC // 128            # 8 m-tiles
GRP = int(os.environ.get("GAT_GRP", "16"))      # j-tiles per phase-A group
CHUNK = int(os.environ.get("GAT_CHUNK", "2"))   # j-tiles per A-mask chunk
ALPHA = 0.2

F32 = mybir.dt.float32
BF16 = mybir.dt.bfloat16
AF = mybir.ActivationFunctionType
BF = ml_dtypes.bfloat16


def build_kernel():
    nc = bacc.Bacc("TRN2", target_bir_lowering=False)

    xt_d = nc.dram_tensor("xt", (FIN, N), BF16, kind="ExternalInput")
    xtl_d = nc.dram_tensor("xt_loc", (FIN, MLOC), BF16, kind="ExternalInput")
    at_d = nc.dram_tensor("at_slab", (N, MLOC), BF16, kind="ExternalInput")
    wext_d = nc.dram_tensor("wext", (FIN, FOUT + 2), BF16, kind="ExternalInput")
    out_d = nc.dram_tensor("out", (MLOC, FOUT), F32, kind="ExternalOutput")
    eyef_d = nc.inline_tensor(np.eye(128, dtype=np.float32), "eyef")

    repeat = int(os.environ.get("GAT_REPEAT", "1"))
    skip = set(os.environ.get("GAT_SKIP", ""))
    nch = NT // CHUNK
    # chunks whose R-mult runs on the (otherwise idle mid-stream) gpsimd
    # engine instead of DVE
    ngp = int(os.environ.get("GAT_NGP", "10"))
    gp_set = set(sorted(range(2, nch - 1, 3))[:ngp])

    with tile.TileContext(nc) as tc:
        with (
            tc.tile_pool(name="const", bufs=1) as constp,
            tc.tile_pool(name="pers", bufs=1) as pers,
            tc.tile_pool(name="at", bufs=int(os.environ.get("GAT_ABUFS", "10"))) as atp,
            tc.tile_pool(name="ut", bufs=int(os.environ.get("GAT_UBUFS", "8"))) as utp,
            tc.tile_pool(name="fin", bufs=3) as finp,
            tc.tile_pool(name="psEF", bufs=1, space="PSUM") as psEF,
            tc.tile_pool(name="psH", bufs=2 if GRP <= 8 else 1, space="PSUM") as psH,
            tc.tile_pool(name="psS", bufs=2, space="PSUM") as psS,
        ):
            # ---------------- constants / inputs ----------------
            wext = constp.tile([128, FOUT + 2], BF16)
            nc.sync.dma_start(out=wext, in_=wext_d.ap())
            xtl = constp.tile([128, MLOC], BF16)
            nc.sync.dma_start(out=xtl, in_=xtl_d.ap())
            ones1 = constp.tile([1, 128], F32)
            nc.vector.memset(ones1, 1.0)

            # A-chunk DMAs alternate SP / Act queues; prefetch three so the
            # wires are busy from t=0 (chunk 0 on Act: ahead of the consts
            # serialized on SP).
            def issue_at_dma():
                a8 = atp.tile([128, CHUNK, MLOC], BF16)
                g = issue_at_dma.next
                issue_at_dma.next = (g + 1) % nch
                src = bass.AP(
                    tensor=at_d, offset=g * CHUNK * 128 * MLOC,
                    ap=[[MLOC, 128], [128 * MLOC, CHUNK], [1, MLOC]],
                )
                eng = nc.scalar if issue_at_dma.flip else nc.sync
                issue_at_dma.flip = not issue_at_dma.flip
                if "d" not in skip:
                    eng.dma_start(out=a8, in_=src)
                else:
                    # tiny DMA keeps the tile allocated, 64x less traffic
                    eng.dma_start(
                        out=a8[:, :, 0:16],
                        in_=bass.AP(tensor=at_d, offset=g * CHUNK * 128 * MLOC,
                                    ap=[[MLOC, 128], [128 * MLOC, CHUNK], [1, 16]]))
                return a8

            issue_at_dma.next = 0
            issue_at_dma.flip = True
            at_q = [issue_at_dma(), issue_at_dma(), issue_at_dma()]
            eyef = constp.tile([128, 128], F32)
            nc.sync.dma_start(out=eyef, in_=eyef_d.ap())

            # X^T in 4 slabs of 16 tiles on the gpsimd queue (idle early)
            xt_sb = pers.tile([128, NT, 128], BF16)
            xt_view = xt_d.ap().rearrange("p (t i) -> p t i", i=128)

            def xt_slab(q):
                nc.gpsimd.dma_start(
                    out=xt_sb[:, q * 16:(q + 1) * 16, :],
                    in_=xt_view[:, q * 16:(q + 1) * 16, :],
                )

            xt_slab(0)

            # ---------------- s1 of local rows -> G1rep ----------------
            # s1locrow [1, 1024] = w1^T @ X_loc^T (two 512-halves)
            s1locrow = pers.tile([1, MLOC], F32)
            for half in range(2):
                srp = psS.tile([1, 512], F32, tag="s")
                nc.tensor.matmul(
                    srp, lhsT=wext[:, FOUT:FOUT + 1],
                    rhs=xtl[:, half * 512:(half + 1) * 512],
                    start=True, stop=True,
                )
                nc.vector.tensor_copy(s1locrow[:, half * 512:(half + 1) * 512], srp)
            # G1row = exp((a-1) s1); replicate down partitions via ones
            # matmul -> G1rep [128, 1024] bf16 (bf16 keeps the TS op in 4x)
            g1row = pers.tile([1, MLOC], F32)
            nc.scalar.activation(g1row, s1locrow, AF.Exp,
                                 bias=0.0, scale=ALPHA - 1.0)
            g1rep = pers.tile([128, MLOC], BF16)
            for half in range(2):
                rp = psS.tile([128, 512], F32, tag="s")
                nc.tensor.matmul(
                    rp, lhsT=ones1,
                    rhs=g1row[:, half * 512:(half + 1) * 512],
                    start=True, stop=True,
                )
                nc.vector.tensor_copy(g1rep[:, half * 512:(half + 1) * 512], rp)
            for q in range(1, 4):
                xt_slab(q)

            # ---------- phase A: h, E2, g2, hE = E2.[h|1] ----------
            hE = pers.tile([128, NT, FOUT + 1], BF16)
            g2all = pers.tile([128, NT], F32)
            for ga in range(NT // GRP):
                g8 = ga * GRP
                hb = psH.tile([128, GRP, FOUT], F32, tag="h")
                s2p = psS.tile([128, GRP, 1], F32, tag="s")
                for t in range(GRP):
                    jt = g8 + t
                    nc.tensor.matmul(
                        hb[:, t, :], lhsT=xt_sb[:, jt, :],
                        rhs=wext[:, 0:FOUT], start=True, stop=True)
                    nc.tensor.matmul(
                        s2p[:, t, :], lhsT=xt_sb[:, jt, :],
                        rhs=wext[:, FOUT + 1:FOUT + 2],
                        start=True, stop=True)
                # E2 = exp(s2), g2 = exp((a-1) s2)  (act reads PSUM)
                he2 = finp.tile([128, GRP, 1], F32)
                nc.scalar.activation(he2, s2p, AF.Exp, bias=0.0, scale=1.0)
                nc.scalar.activation(
                    g2all[:, g8:g8 + GRP].rearrange("p (g o) -> p g o", o=1),
                    s2p, AF.Exp, bias=0.0, scale=ALPHA - 1.0)
                # stage h into SBUF so gpsimd can scale it
                hbs = finp.tile([128, GRP, FOUT], BF16)
                nc.scalar.activation(hbs, hb, AF.Copy, bias=0.0, scale=1.0)
                for t in range(GRP):
                    jt = g8 + t
                    nc.gpsimd.tensor_scalar(
                        hE[:, jt, 0:FOUT], hbs[:, t, :], he2[:, t, :],
                        None, AluOpType.mult)
                nc.gpsimd.tensor_copy(
                    hE[:, g8:g8 + GRP, FOUT:FOUT + 1], he2)

            # ---------------- phase B ----------------
            psEFt = psEF.tile([FOUT + 1, MLOC], F32, tag="EF")
            if "m" in skip:
                nc.vector.memset(psEFt, 0.0)
            total = repeat * nch
            tot = NT * 2
            for step in range(total):
                g = step % nch
                if g == 0:
                    cnt = [0, 0]
                a8 = at_q[step]
                if step + 3 < total:
                    at_q.append(issue_at_dma())
                u8 = utp.tile([128, CHUNK, MLOC], BF16)
                if "s" not in skip:
                    for k in range(CHUNK):
                        jt = g * CHUNK + k
                        nc.vector.tensor_scalar(
                            u8[:, k, :], g1rep, g2all[:, jt:jt + 1], 1.0,
                            AluOpType.mult, AluOpType.max)
                # R = A . U in place on u8
                per_tile = step == total - 1
                eng = nc.gpsimd if g in gp_set else nc.vector
                for k0 in (range(CHUNK) if per_tile else [0]):
                    kw = 1 if per_tile else CHUNK
                    if "t" not in skip:
                        eng.tensor_tensor(
                            u8[:, k0:k0 + kw, :], u8[:, k0:k0 + kw, :],
                            a8[:, k0:k0 + kw, :], AluOpType.mult)
                    if "m" in skip:
                        continue
                    for k in range(k0, k0 + kw):
                        jt = g * CHUNK + k
                        for half in range(2):
                            nc.tensor.matmul(
                                psEFt[:, half * 512:(half + 1) * 512],
                                lhsT=hE[:, jt, :],
                                rhs=u8[:, k, half * 512:(half + 1) * 512],
                                start=cnt[half] == 0,
                                stop=cnt[half] == tot - 2,
                            )
                            cnt[half] += 2

            # ---------------- finalize ----------------
            oEF = pers.tile([FOUT + 1, MLOC], F32)
            nc.vector.tensor_copy(oEF, psEFt)
            for t in range(MT):
                trE = psS.tile([128, FOUT + 1], F32, tag="s")
                nc.tensor.transpose(
                    trE, oEF[:, t * 128:(t + 1) * 128],
                    eyef[0:FOUT + 1, 0:FOUT + 1])
                rec = finp.tile([128, 1], F32)
                nc.vector.reciprocal(rec, trE[:, FOUT:FOUT + 1])
                fin = finp.tile([128, FOUT], F32)
                nc.vector.tensor_scalar(fin, trE[:, 0:FOUT], rec, None,
                                        AluOpType.mult)
                nc.sync.dma_start(
                    out=out_d.ap()[t * 128:(t + 1) * 128, :], in_=fin)

    nc.compile()
    return nc


_NC = None


def kernel(X, A, W, a1, a2):
    global _NC
    X = np.asarray(X, dtype=np.float32)
    A = np.asarray(A)
    W = np.asarray(W, dtype=np.float32)
    a1 = np.asarray(a1, dtype=np.float32)
    a2 = np.asarray(a2, dtype=np.float32)

    xt = np.ascontiguousarray(X.T).astype(BF)
    wext = np.ascontiguousarray(
        np.concatenate([W, (W @ a1)[:, None], (W @ a2)[:, None]], axis=1)
    ).astype(BF)
    A_bf = A.astype(BF)

    if _NC is None:
        _NC = build_kernel()
    nc = _NC
    in_maps = []
    for c in range(NCORES):
        rows = slice(c * MLOC, (c + 1) * MLOC)
        in_maps.append({
            "xt": xt,
            "xt_loc": np.ascontiguousarray(xt[:, rows]),
            "at_slab": np.ascontiguousarray(A_bf[rows].T),
            "wext": wext,
        })
    res = run_bass_kernel_spmd(nc, in_maps, core_ids=list(range(NCORES)))
    return np.concatenate([r["out"] for r in res.results], axis=0)


if __name__ == "__main__":
    rng = np.random.default_rng(0)
    X = rng.standard_normal((N, FIN), dtype=np.float32)
    A = rng.integers(0, 2, (N, N), dtype=np.int32)
    W = (rng.standard_normal((FIN, FOUT), dtype=np.float32) * 0.05)
    a1 = (rng.standard_normal((FOUT,), dtype=np.float32) * 0.05)
    a2 = (rng.standard_normal((FOUT,), dtype=np.float32) * 0.05)
    out = kernel(X=X, A=A, W=W, a1=a1, a2=a2)
    h = X @ W
    s1 = h @ a1
    s2 = h @ a2
    e = s1[:, None] + s2[None, :]
    e = np.where(e > 0, e, ALPHA * e)
    att = np.where(A > 0, np.exp(e - e.max(1, keepdims=True)), 0.0)
    att = att / att.sum(1, keepdims=True)
    ref = att @ h
    err = np.abs(out - ref).max() / np.abs(ref).max()
    print("rel err (max-abs scaled):", err)
